# revision 58
# baseline (speedup 1.0000x reference)
"""Trainium2 Bass/Tile kernel: two chained VALID 3x3 convolutions.

    x  [N,3,256,256] --conv(w1)--> h [N,64,254,254] --conv(w2)--> out [N,128,252,252]

Data-parallel over 8 NeuronCores: batch N=16 -> 2 images per core, conv
weights replicated.  Per core the convs are computed as implicit GEMMs on the
tensor engine:

  conv1: contraction over C0*3*3=27 on SBUF partitions (im2col buffer built
         with 9 strided DMAs), one matmul per 2-row output chunk.
  conv2: contraction over C1*9=576.  h is stored doubled in SBUF: partitions
         0:64 hold h rows, partitions 64:128 the same rows shifted down by
         one.  A K=128 matmul computes a pair of row-taps (di,dj)+(di+1,dj)
         at once: 3 pairs + 3 singles = 6 matmuls per 2-row chunk.  The
         singles (taps (2,dj)) are zero-top-padded K=128 passes on the
         shifted half so the PE never pays 64<->128 tile-config switches;
         conv2 output is written to HBM as bf16 (host upcasts), halving
         output DMA traffic.

Perf structure (v2): the PE HAM throttle holds the array at 1.2 GHz unless the
matmul stream is gap-free, so the whole kernel is a single dense PE stream:
conv1 matmuls for strip s+2 are interleaved in small bursts between conv2
chunk matmuls of strip s.  conv1 PSUM eviction runs on ScalarE, conv2
eviction on VectorE.  The h row-shift copy is done strip-at-a-time with 4
coarse SBUF->SBUF DMAs; conv2 output is staged in SBUF and written to HBM in
6-row (6 KB/partition) pieces to cut DMA packet overhead.
"""

from contextlib import ExitStack

import ml_dtypes
import numpy as np

import concourse.bass as bass
import concourse.mybir as mybir
import concourse.tile as tile
import concourse.bass_utils as bass_utils
from concourse import bacc

N_CORES = 8
FULL_N = 16
C0, C1, C2 = 3, 64, 128

MODE = "bf16"


def _mm_dt():
    return mybir.dt.bfloat16 if MODE == "bf16" else mybir.dt.float32r


def _np_dt():
    return ml_dtypes.bfloat16 if MODE == "bf16" else np.float32


class Geom:
    def __init__(self, npc, h0, w0, ty):
        self.npc = npc          # images per core
        self.h0, self.w0 = h0, w0
        self.h1, self.w1 = h0 - 2, w0 - 2
        self.h2, self.w2 = h0 - 4, w0 - 4
        self.ty = ty            # conv2 output rows per strip
        assert ty % 2 == 0 and self.h2 % ty == 0
        self.strips_per_img = self.h2 // ty
        self.nstrips = npc * self.strips_per_img
        self.nc1 = (ty + 2) // 2            # conv1 2-row chunks per strip
        self.nc2 = ty // 2                  # conv2 2-row chunks per strip


GEOM = Geom(npc=FULL_N // N_CORES, h0=256, w0=256, ty=36)

# conv2 chunks are emitted in blocks of BLK chunks; after blocks 0..len-1 a
# burst of conv1 chunks (for strip s+2) of the given size is emitted.
BLK = 3
CONV1_BURSTS = [4, 4, 4, 4, 3, 0]           # sums to nc1=19, len = nc2//BLK


def _emit(ctx: ExitStack, tc: tile.TileContext, g: Geom, out, x, w1t, w2p, w2s,
          mm_dt):
    nc = tc.nc
    f32 = mybir.dt.float32
    TY, W1, W2 = g.ty, g.w1, g.w2
    NB = g.nc2 // BLK                       # conv2 blocks per strip
    assert len(CONV1_BURSTS) == NB and sum(CONV1_BURSTS) == g.nc1

    wpool = ctx.enter_context(tc.tile_pool(name="weights", bufs=1))
    b1pool = ctx.enter_context(tc.tile_pool(name="b1", bufs=4))
    hpool = ctx.enter_context(tc.tile_pool(name="h", bufs=3))
    opool = ctx.enter_context(tc.tile_pool(name="o2", bufs=8))
    ps1 = ctx.enter_context(tc.tile_pool(name="ps1", bufs=4, space="PSUM"))
    ps2 = ctx.enter_context(tc.tile_pool(name="ps2", bufs=4, space="PSUM"))

    w1t_sb = wpool.tile([27, C1], mm_dt)
    nc.sync.dma_start(w1t_sb[:], w1t)
    # one fully-contiguous weight tile per conv2 tap column (FWL-friendly)
    w2p_sb = []
    w2s_sb = []
    for dj in range(3):
        wp = wpool.tile([128, C2], mm_dt, tag=f"w2p{dj}")
        nc.sync.dma_start(wp[:], w2p[dj])
        w2p_sb.append(wp)
        ws = wpool.tile([128, C2], mm_dt, tag=f"w2s{dj}")
        nc.sync.dma_start(ws[:], w2s[dj])
        w2s_sb.append(ws)

    def strip_of(s):
        n, k = divmod(s, g.strips_per_img)
        return n, k * TY

    B1_tiles = {}
    H_tiles = {}

    def emit_x_load(s, split=False):
        n, y0 = strip_of(s)
        B1 = b1pool.tile([27, TY + 2, W1], mm_dt, tag="b1", name=f"B1_{s}")
        B1_tiles[s] = B1
        # split=True loads each tap in two row-halves so the first conv1
        # chunks only wait for half the packets (prologue latency).
        bounds = [0, 20, TY + 2] if split else [0, TY + 2]
        for a, b in zip(bounds, bounds[1:]):
            for t9 in range(9):
                di, dj = divmod(t9, 3)
                nc.sync.dma_start(
                    B1[3 * t9:3 * t9 + 3, a:b, :],
                    x[n, :, y0 + di + a:y0 + di + b, dj:dj + W1])

    def emit_conv1_chunk(s, j, eng):
        """conv1 chunk j (h rows 2j:2j+2 of the strip) -> H[0:64]."""
        if j == 0:
            H_tiles[s] = hpool.tile([128, TY + 2, W1], mm_dt, tag="h",
                                    name=f"H{s}")
        H = H_tiles[s]
        B1 = B1_tiles[s]
        r = 2 * j
        P1 = ps1.tile([C1, 2, W1], f32, tag="p1")
        nc.tensor.matmul(P1[:], w1t_sb[:], B1[:, r:r + 2, :],
                         start=True, stop=True)
        if eng == "act":
            nc.scalar.copy(H[0:C1, r:r + 2, :], P1[:])
        else:
            nc.vector.tensor_copy(H[0:C1, r:r + 2, :], P1[:])

    def emit_shift(s):
        """Row-shifted copy H[64:128, r] = h row r+1, strip at a time."""
        H = H_tiles[s]
        R = TY + 1                           # 37 rows to fill
        bounds = [0, 10, 20, 29, R]
        for a, b in zip(bounds, bounds[1:]):
            nc.sync.dma_start(H[C1:128, a:b, :], H[0:C1, a + 1:b + 1, :])

    def emit_conv2_block(s, k):
        """conv2 chunks 3k..3k+2 of strip s -> staged 6-row piece -> HBM."""
        n, y0 = strip_of(s)
        H = H_tiles[s]
        OS = opool.tile([C2, 2 * BLK, W2], mm_dt, tag="os")
        # all K=128 pair matmuls of the block first, then all K=64 singles:
        # the PE pays ~90ns per 128-row <-> 64-row tile-config switch, so 2
        # switches per block instead of 2 per chunk.  Accumulation groups
        # interleave across banks (hardware-legal; skip the sim group check).
        P2s = []
        for c in range(BLK):
            t = (BLK * k + c) * 2
            P2 = ps2.tile([C2, 2, W2], f32, tag="p2", name=f"P2_{c}")
            P2s.append((t, P2))
            for dj in range(3):              # pairs: taps (0,dj)+(1,dj)
                nc.tensor.matmul(P2[:], w2p_sb[dj][:],
                                 H[:, t:t + 2, dj:dj + W2],
                                 start=(dj == 0), stop=False,
                                 skip_group_check=True)
        for c in range(BLK):
            t, P2 = P2s[c]
            for dj in range(3):
                # singles: tap (2,dj) as zero-top-padded K=128 passes on
                # H's shifted half -- same cycles as K=64 but no 64<->128
                # PE tile-config switch (~90ns each, 2 per block).
                nc.tensor.matmul(P2[:], w2s_sb[dj][:],
                                 H[:, t + 1:t + 3, dj:dj + W2],
                                 start=False, stop=(dj == 2),
                                 skip_group_check=True)
            nc.vector.tensor_copy(OS[:, 2 * c:2 * c + 2, :], P2[:])
        y = y0 + 2 * BLK * k
        # two chains so two DMA engines drain the output in parallel
        nc.sync.dma_start(out[n, 0:C1, y:y + 2 * BLK, :], OS[0:C1])
        nc.sync.dma_start(out[n, C1:C2, y:y + 2 * BLK, :], OS[C1:C2])

    S = g.nstrips
    # prologue: im2col for strips 0-2, conv1 for strips 0 and 1 (evictions
    # alternate DVE/ACT so the PE stream has no eviction stalls), shift(0)
    # overlapping conv1(1).
    emit_x_load(0, split=True)
    emit_x_load(1, split=True)
    emit_x_load(2)
    for j in range(g.nc1):
        emit_conv1_chunk(0, j, "act" if j % 2 else "dve")
    emit_shift(0)
    for j in range(g.nc1):
        emit_conv1_chunk(1, j, "act" if j % 2 else "dve")

    # steady state: phase s = conv2(s) with conv1(s+2) interleaved in bursts
    for s in range(S):
        if s + 3 < S:
            emit_x_load(s + 3)
        if s + 1 < S:
            emit_shift(s + 1)
        u = s + 2                            # conv1 target strip
        j0 = 0
        for k in range(NB):
            emit_conv2_block(s, k)
            if u < S:
                for j in range(j0, j0 + CONV1_BURSTS[k]):
                    emit_conv1_chunk(u, j, "act")
                j0 += CONV1_BURSTS[k]


def build(g: Geom = GEOM, mm_dt=None):
    if mm_dt is None:
        mm_dt = _mm_dt()
    nc = bacc.Bacc("TRN2", target_bir_lowering=False, debug=False,
                   num_devices=N_CORES)
    f32 = mybir.dt.float32
    x = nc.dram_tensor("x", [g.npc, C0, g.h0, g.w0], mm_dt,
                       kind="ExternalInput").ap()
    w1t = nc.dram_tensor("w1t", [27, C1], mm_dt, kind="ExternalInput").ap()
    w2p = nc.dram_tensor("w2p", [3, 128, C2], mm_dt, kind="ExternalInput").ap()
    w2s = nc.dram_tensor("w2s", [3, 128, C2], mm_dt, kind="ExternalInput").ap()
    out = nc.dram_tensor("out", [g.npc, C2, g.h2, g.w2], mm_dt,
                         kind="ExternalOutput").ap()
    with tile.TileContext(nc) as tc:
        with ExitStack() as ctx:
            _emit(ctx, tc, g, out, x, w1t, w2p, w2s, mm_dt)
    nc.compile()
    return nc


def host_round(a: np.ndarray) -> np.ndarray:
    """Cast fp32 to the matmul storage dtype (bf16 cast, or tf32 rounding)."""
    a = np.ascontiguousarray(a, dtype=np.float32)
    if MODE == "bf16":
        return a.astype(ml_dtypes.bfloat16)
    b = a.view(np.uint32).copy()
    b += 0xFFF + ((b >> 13) & 1)
    b &= np.uint32(0xFFFFE000)
    return b.view(np.float32)


def pack_weights(w1: np.ndarray, w2: np.ndarray):
    """Host-side repack so every device DMA is contiguous.

    w1t[p, o] = w1[o, c, di, dj] with p = (di*3+dj)*3 + c  (matches im2col)
    w2p[dj, k, o]: k<64 -> w2[o, k, 0, dj]; k>=64 -> w2[o, k-64, 1, dj]
    w2s[dj, c, o] = w2[o, c, 2, dj]
    """
    w1 = np.ascontiguousarray(np.asarray(w1), dtype=np.float32)
    w2 = np.ascontiguousarray(np.asarray(w2), dtype=np.float32)
    w1t = np.ascontiguousarray(w1.transpose(2, 3, 1, 0).reshape(27, C1))
    w2p = np.empty((3, 128, C2), np.float32)
    w2p[:, :C1] = w2[:, :, 0, :].transpose(2, 1, 0)
    w2p[:, C1:] = w2[:, :, 1, :].transpose(2, 1, 0)
    w2s = np.zeros((3, 128, C2), np.float32)
    w2s[:, C1:] = w2[:, :, 2, :].transpose(2, 1, 0)
    return host_round(w1t), host_round(w2p), host_round(w2s)


_NC_CACHE: dict = {}


def _get_nc():
    key = ("main", MODE, GEOM.ty)
    if key not in _NC_CACHE:
        _NC_CACHE[key] = build()
    return _NC_CACHE[key]


def run(x, w1, w2, trace: bool = False):
    """Shard, run on 8 cores, gather.  Returns (out, BassKernelResults)."""
    x = np.ascontiguousarray(np.asarray(x), dtype=np.float32)
    assert x.shape == (FULL_N, C0, GEOM.h0, GEOM.w0), x.shape
    w1t, w2p, w2s = pack_weights(w1, w2)
    xs = host_round(x).reshape(N_CORES, GEOM.npc, C0, GEOM.h0, GEOM.w0)
    in_maps = [
        {"x": np.ascontiguousarray(xs[c]), "w1t": w1t, "w2p": w2p, "w2s": w2s}
        for c in range(N_CORES)
    ]
    nc = _get_nc()
    res = bass_utils.run_bass_kernel_spmd(
        nc, in_maps, core_ids=list(range(N_CORES)), trace=trace)
    out = np.concatenate(
        [np.asarray(r["out"], dtype=np.float32) for r in res.results], axis=0)
    return out, res


def kernel(x, w1, w2):
    out, _ = run(x, w1, w2, trace=False)
    return out

